# revision 1
# baseline (speedup 1.0000x reference)
"""Trainium2 Bass kernel for nn_Attention_12000138625343.

Full multi-head attention layer (B=2, S=2048, E=1024, H=16, hd=64, interleaved
RoPE on q/k, non-causal softmax) run tensor-parallel over 8 NeuronCores:

  - heads sharded 2-per-core (w1 columns / qkv projection sharded),
  - x replicated and host-cast to bf16, passed pre-transposed [E, B*S] so the
    contraction dim lands on SBUF partitions; w1 slices and w2 also bf16
    (halves the ~23 MB input DMA stream; ~0.5% added error vs the 2e-2 gate),
  - scores computed transposed [k, q] in f32r; the two heads' K=64 score
    matmuls are packed into disjoint PE row-groups (concurrent), one exp
    instruction covers both heads' [128, 1024] PSUM block,
  - the v projection runs in its fast transposed orientation (N=512) and is
    flipped back to [k, hd] by PE transposes against an identity,
  - attn@v accumulates rolling per k-chunk with a ones-column appended to v
    producing the softmax denominator; the divide runs off the TensorEngine
    (reciprocal_approx_fast on DVE -- custom op, base partition 0 only --
    + DRAM-bounce broadcast DMA + DVE multiply); the final q-tile runs the
    two heads' chains stage-parallel across ACT/DVE and bounces over the
    sync+scalar rings, keeping the PE queue (busy with recv-gated
    projections) out of the collective's gate,
  - batch-0 attention starts as soon as k(rt0)+q(rt0) are projected; the
    remaining batch-0 qkv chains and then batch-1's chains are dribbled into
    the attention unit loop on a deadline schedule (all consumers of x tile
    n emitted before any consumer of tile n+3; xtp pool has 3 buffers),
  - both DGE rings process descriptors serially, so the sync ring carries
    only x halves / wq / p2 and the divide+A2A-staging DMAs, the gpsimd ring
    the rest; output writes go on the scalar ring,
  - four AllToAlls (one per batch-half, each gated by that half's last
    softmax divide, bf16 payload) convert the head sharding of the attention
    output o^T into row sharding; one warmup collective at t=0 absorbs the
    CC cold start (cross-core rendezvous, 50-200us run-to-run),
  - each core owns 2 x 128 rows of each batch; host reassembles.

Measured on the fixture: ~310-320us (CC cold-start adds run-to-run variance),
rel err ~5.2e-3. The PE runs at the ~1.2 GHz mid p-state throughout (full
2.4 GHz was never observed on this part, for any dtype or stream density).
"""

import math

import numpy as np

import concourse.bass as bass
import concourse.mybir as mybir
import concourse.tile as tile
from concourse import bacc
from concourse.bass_utils import run_bass_kernel_spmd
from concourse.masks import make_identity

B, S, E, H = 2, 2048, 1024, 16
HD = E // H  # 64
BASE = 10000.0
N_CORES = 8
HPC = H // N_CORES       # heads per core = 2
R = B * S                # 4096 flattened rows
RT = 512                 # rows per r-tile
NEC = E // 128           # 8 e-chunks of 128
QT = 512                 # q columns per q-tile
N_QT = S // QT           # 4 q-tiles per batch
KC = 128                 # k rows per k-chunk
N_KC = S // KC           # 16 k-chunks per batch
RPB = S // N_CORES       # rows per core per batch = 256

F32 = mybir.dt.float32
F32R = mybir.dt.float32r
BF16 = mybir.dt.bfloat16
EXPF = mybir.ActivationFunctionType.Exp

_COMPILED = {}


def _build_nc():
    nc = bacc.Bacc("TRN2", target_bir_lowering=False, debug=False,
                   num_devices=N_CORES)

    xT = nc.dram_tensor("xT", [E, R], BF16, kind="ExternalInput").ap()
    wqT = nc.dram_tensor("wqT", [E, 128], BF16, kind="ExternalInput").ap()
    wkT = nc.dram_tensor("wkT", [E, 128], BF16, kind="ExternalInput").ap()
    wvT = nc.dram_tensor("wvT", [E, 128], BF16, kind="ExternalInput").ap()
    w2T = nc.dram_tensor("w2T", [E, E], BF16, kind="ExternalInput").ap()
    cosT = nc.dram_tensor("cosT", [128, S], F32, kind="ExternalInput").ap()
    sinT = nc.dram_tensor("sinT", [128, S], F32, kind="ExternalInput").ap()
    p2T = nc.dram_tensor("p2T", [128, 128], F32, kind="ExternalInput").ap()
    out = nc.dram_tensor("out", [2 * RPB, E], F32, kind="ExternalOutput").ap()

    with tile.TileContext(nc) as tc:
        _emit(tc, nc, xT, wqT, wkT, wvT, w2T, cosT, sinT, p2T, out)
    nc.compile()
    return nc


def _emit(tc, nc, xT, wqT, wkT, wvT, w2T, cosT, sinT, p2T, out):
    import contextlib
    ctx = contextlib.ExitStack()
    consts = ctx.enter_context(tc.tile_pool(name="consts", bufs=1))
    xtp = ctx.enter_context(tc.tile_pool(name="xtp", bufs=3))
    qkp = ctx.enter_context(tc.tile_pool(name="qkp", bufs=1))
    rawp = ctx.enter_context(tc.tile_pool(name="rawp", bufs=2))
    tmpp = ctx.enter_context(tc.tile_pool(name="tmpp", bufs=2))
    vp = ctx.enter_context(tc.tile_pool(name="vp", bufs=1))
    pp = ctx.enter_context(tc.tile_pool(name="pp", bufs=7))
    smallp = ctx.enter_context(tc.tile_pool(name="smallp", bufs=2))
    dramp = ctx.enter_context(tc.tile_pool(name="dramp", bufs=1, space="DRAM"))
    # PSUM budget (8 banks): qkv-shared 2 + sps 2 x 2 + av 2 = 8
    ps_qkv = ctx.enter_context(tc.tile_pool(name="ps_qkv", bufs=2, space="PSUM"))
    ps_sps = ctx.enter_context(tc.tile_pool(name="ps_sps", bufs=2, space="PSUM"))
    ps_av = ctx.enter_context(tc.tile_pool(name="ps_av", bufs=2, space="PSUM"))

    # ---- fabric warmups first: three no-data-dep collectives on garbage
    # DRAM absorb the CC cold start (~55us) and the bandwidth ramp ----
    cwu = [dramp.tile([N_CORES, 8], F32, tag=f"cwu{i}", name=f"cwu{i}")
           for i in range(2)]
    nc.gpsimd.collective_compute(
        "AllToAll", mybir.AluOpType.bypass,
        replica_groups=[list(range(N_CORES))],
        ins=[cwu[0].opt()], outs=[cwu[1].opt()])

    # ---- tiny constants first: the identity (gpsimd iota) must precede the
    # xt posts on the gpsimd ring or the v-transposes deadlock against a
    # blocked xt DMA ----
    ones_f32 = consts.tile([128, 64], F32, tag="ones32", name="ones_f32")
    nc.vector.memset(ones_f32[:], 1.0)
    ones_r = consts.tile([1, 64], F32R, tag="onesr", name="ones_r")
    nc.vector.tensor_copy(ones_r[:], ones_f32[0:1, 0:64])
    id_sb = consts.tile([128, 128], F32, tag="idm", name="id_sb")
    make_identity(nc, id_sb[:])

    # ---- weight/x loads, k-chain inputs first so scores can start early.
    # Each batch-0 x r-tile is split across the gpsimd and sync DMA queues
    # (e-chunks 0-3 / 4-7) so both rings pull HBM concurrently; cos/sin are
    # loaded per 512-column chunk just in time for each r-tile's RoPE ----
    xTr = xT.rearrange("(c p) r -> p c r", p=128)
    wk_all = consts.tile([128, NEC, 128], BF16, tag="wk", name="wk_all")
    nc.gpsimd.dma_start(out=wk_all[:], in_=wkT.rearrange("(c p) f -> p c f", p=128))
    wq_all = consts.tile([128, NEC, 128], BF16, tag="wq", name="wq_all")
    nc.sync.dma_start(out=wq_all[:],
                  in_=wqT.rearrange("(c p) f -> p c f", p=128))
    xts = {}
    cos_sb = consts.tile([128, S], F32, tag="cos", name="cos_sb")
    sin_sb = consts.tile([128, S], F32, tag="sin", name="sin_sb")
    p2_sb = consts.tile([128, 128], F32R, tag="p2", name="p2_sb")
    wv_all = consts.tile([128, NEC, 128], BF16, tag="wv", name="wv_all")

    def post_xt_split(rt, four=False):
        t = xtp.tile([128, NEC, RT], BF16, tag="xt", name=f"xt_{rt}")
        c0, c1 = rt * RT, (rt + 1) * RT
        if four:
            # first tiles gate the whole pipeline: pull them over four DGE
            # rings at once (each ring drains its descriptors serially)
            nc.gpsimd.dma_start(out=t[:, 0:3, :], in_=xTr[:, 0:3, c0:c1])
            nc.sync.dma_start(out=t[:, 3:5, :], in_=xTr[:, 3:5, c0:c1])
            nc.scalar.dma_start(out=t[:, 5:NEC, :], in_=xTr[:, 5:NEC, c0:c1])
        else:
            nc.gpsimd.dma_start(out=t[:, 0:4, :], in_=xTr[:, 0:4, c0:c1])
            nc.sync.dma_start(out=t[:, 4:NEC, :], in_=xTr[:, 4:NEC, c0:c1])
        xts[rt] = t
        return t

    def cossin(i):
        nc.gpsimd.dma_start(out=cos_sb[:, i * RT:(i + 1) * RT],
                            in_=cosT[:, i * RT:(i + 1) * RT])
        nc.gpsimd.dma_start(out=sin_sb[:, i * RT:(i + 1) * RT],
                            in_=sinT[:, i * RT:(i + 1) * RT])

    # sync ring carries only x halves (plus wq/p2): both DGE rings process
    # their descriptors serially, so RoPE tables must not delay x tiles
    post_xt_split(0, four=True)
    nc.sync.dma_start(out=p2_sb[:], in_=p2T[:, :].bitcast(F32R))
    nc.gpsimd.dma_start(out=wv_all[:], in_=wvT.rearrange("(c p) f -> p c f", p=128))
    cossin(0)
    post_xt_split(1, four=True)
    cossin(1)
    post_xt_split(2)
    cossin(2)
    post_xt_split(3)
    cossin(3)

    # A2A buffers, one pair per (batch, half): [8 chunks, 128 e-rows, 128 rows]
    send_d = {(b, hf): dramp.tile([N_CORES, 128, 128], BF16, name=f"send{b}{hf}")
              for b in range(B) for hf in range(2)}
    recv_d = {(b, hf): dramp.tile([N_CORES, 128, 128], BF16, name=f"recv{b}{hf}")
              for b in range(B) for hf in range(2)}

    qT_sb, kT_sb, v_sb = {}, {}, {}
    w2_sb = {}

    def emit_xt_load(rt):
        # batch-1 tiles load on gpsimd only: their posts block on xtp pool
        # reuse, and the sync queue must stay clear for the divide DMAs
        if rt in xts:
            return xts[rt]
        t = xtp.tile([128, NEC, RT], BF16, tag="xt", name=f"xt_{rt}")
        nc.gpsimd.dma_start(out=t[:], in_=xTr[:, :, rt * RT:(rt + 1) * RT])
        xts[rt] = t
        return t

    def qk_chain(kind, rt, dribbled):
        """Two closures emitting the q- or k-projection (+RoPE) for r-tile
        rt. Dribbled chains evict on DVE to keep ACT free for exp."""
        b, st = rt // N_QT, (rt % N_QT) * RT
        w_all = wq_all if kind == "q" else wk_all
        if b not in qT_sb:
            qT_sb[b] = qkp.tile([128, S], F32R, tag=f"qT{b}", name=f"qT{b}")
            kT_sb[b] = qkp.tile([128, S], F32R, tag=f"kT{b}", name=f"kT{b}")
        dst = qT_sb[b] if kind == "q" else kT_sb[b]
        state = {}

        def emit_a():
            xt = xts[rt]
            acc = ps_qkv.tile([128, RT], F32, tag="qkv", name=f"{kind}acc{rt}")
            for ec in range(4):
                nc.tensor.matmul(acc[:], w_all[:, ec, :], xt[:, ec, :],
                                 start=(ec == 0), stop=False)
            state["acc"] = acc

        def emit_b():
            xt = xts[rt]
            acc = state.pop("acc")
            for ec in range(4, NEC):
                nc.tensor.matmul(acc[:], w_all[:, ec, :], xt[:, ec, :],
                                 start=False, stop=(ec == NEC - 1))
            raw = rawp.tile([128, RT], F32R, tag="raw", name=f"{kind}raw{rt}")
            if dribbled:
                nc.vector.tensor_copy(raw[:], acc[:])
            else:
                nc.scalar.copy(raw[:], acc[:])
            rot = ps_qkv.tile([128, RT], F32, tag="qkv", name=f"{kind}rot{rt}")
            nc.tensor.matmul(rot[:], p2_sb[:], raw[:], start=True, stop=True)
            t1 = tmpp.tile([128, RT], F32, tag="ropet", name=f"{kind}t1_{rt}")
            nc.vector.tensor_mul(t1[:], raw[:].bitcast(F32),
                                 cos_sb[:, st:st + RT])
            t2 = tmpp.tile([128, RT], F32, tag="ropet", name=f"{kind}t2_{rt}")
            nc.vector.tensor_mul(t2[:], rot[:], sin_sb[:, st:st + RT])
            nc.vector.tensor_add(dst[:, st:st + RT], t1[:], t2[:])
        return [emit_a, emit_b]

    def v_chains(rt, dribbled):
        """Four closures for the v projection of r-tile rt: two matmul halves
        in transposed orientation, two transpose-back pairs."""
        b = rt // N_QT
        vstate = {}

        def head(half):
            def emit():
                xt = xts[rt]
                if half == 0:
                    vacc = ps_qkv.tile([128, RT], F32, tag="qkv",
                                       name=f"vTacc{rt}")
                    vstate["ps"] = vacc
                vacc = vstate["ps"]
                for ec in range(4 * half, 4 * half + 4):
                    nc.tensor.matmul(vacc[:], wv_all[:, ec, :], xt[:, ec, :],
                                     start=(ec == 0), stop=(ec == NEC - 1))
                if half == 1:
                    vts = rawp.tile([128, RT], F32, tag="raw", name=f"vts{rt}")
                    if dribbled:
                        nc.vector.tensor_copy(vts[:], vstate.pop("ps")[:])
                    else:
                        nc.scalar.copy(vts[:], vstate.pop("ps")[:])
                    vstate["sb"] = vts
            return emit

        def tail(pair):
            def emit():
                vts = vstate["sb"]
                for sub in (2 * pair, 2 * pair + 1):
                    vtr = ps_qkv.tile([128, 128], F32, tag="qkv",
                                      name=f"vtr{rt}_{sub}")
                    nc.tensor.transpose(
                        vtr[:], vts[:, sub * 128:(sub + 1) * 128], id_sb[:])
                    kc = (rt % N_QT) * 4 + sub
                    for h in range(HPC):
                        vt = vp.tile([128, 65], F32R, tag=f"v{b}{h}{kc}",
                                     name=f"v{b}{h}{kc}")
                        nc.vector.tensor_copy(vt[:, 0:64],
                                              vtr[:, h * 64:(h + 1) * 64])
                        nc.vector.tensor_copy(vt[:, 64:65], ones_f32[:, 0:1])
                        v_sb[(b, h, kc)] = vt
            return emit

        return [head(0), head(1), tail(0), tail(1)]

    def proj_chains(b, hf):
        """Output projection for my 128 rows of (batch b, half hf)."""
        state0 = {}
        def get_recv():
            if "t" not in state0:
                t = xtp.tile([128, NEC, 128], BF16, tag="recv", bufs=2,
                             name=f"recv{b}{hf}")
                rr = recv_d[(b, hf)].rearrange("c p r -> p c r")
                nc.gpsimd.dma_start(out=t[:, 0:4, :], in_=rr[:, 0:4, :])
                nc.gpsimd.dma_start(out=t[:, 4:NEC, :], in_=rr[:, 4:NEC, :])
                state0["t"] = t
            return state0["t"]
        chains = []
        for rblk in [hf]:
            for ft in range(2):
                state = {}
                def emit_a(rblk=rblk, ft=ft, state=state):
                    recv_sb = get_recv()
                    ops = ps_qkv.tile([128, 512], F32, tag="qkv",
                                      name=f"ops{b}_{rblk}_{ft}")
                    for ec in range(4):
                        nc.tensor.matmul(
                            ops[:],
                            recv_sb[:, ec, :],
                            w2_sb[0][:, ec, ft * 512:(ft + 1) * 512],
                            start=(ec == 0), stop=False)
                    state["ops"] = ops
                def emit_b(rblk=rblk, ft=ft, state=state):
                    recv_sb = get_recv()
                    ops = state.pop("ops")
                    for ec in range(4, NEC):
                        nc.tensor.matmul(
                            ops[:],
                            recv_sb[:, ec, :],
                            w2_sb[0][:, ec, ft * 512:(ft + 1) * 512],
                            start=False, stop=(ec == NEC - 1))
                    ot = tmpp.tile([128, 512], F32, tag="ropet",
                                   name=f"ot{b}_{rblk}_{ft}")
                    if b == 0:
                        nc.vector.tensor_copy(ot[:], ops[:])
                    else:
                        nc.scalar.copy(ot[:], ops[:])
                    ob = 2 * b + rblk
                    nc.scalar.dma_start(
                        out=out[ob * 128:(ob + 1) * 128,
                                ft * 512:(ft + 1) * 512],
                        in_=ot[:])
                chains.append(emit_a)
                chains.append(emit_b)
        return chains

    def emit_divide(b, qt, avs):
        """Divide by the softmax denominator (row 64 of av) and stage into
        the A2A send buffer. PE-free: fast DVE reciprocal + DRAM-bounce
        broadcast DMA; one 3D DMA stages all four destination chunks."""
        last = (b == B - 1 and qt == N_QT - 1)
        hf = qt // 2
        j0 = 4 * (qt % 2)

        def send(h, odiv):
            # q-tile qt covers s in [512qt, 512qt+512): half hf = qt // 2,
            # destination cores j = 4*(qt%2) .. +4, 128 columns each
            nc.sync.dma_start(
                out=send_d[(b, hf)].rearrange("j p r -> p j r")
                    [h * 64:(h + 1) * 64, j0:j0 + 4, :],
                in_=odiv[:].rearrange("p (j r) -> p j r", j=4))

        if last:
            # stage-parallel: h0 evicts/broadcasts on ACT while h1 uses DVE,
            # so the tail collective's gate closes as early as possible
            oraws, bcss, odivs = [], [], []
            for h in range(HPC):
                ecopy = nc.scalar.copy if h == 0 else nc.vector.tensor_copy
                oraw = smallp.tile([64, QT], F32, tag="oraw",
                                   name=f"oraw{b}{h}{qt}")
                ecopy(oraw[:], avs[h][0:64, :])
                den = smallp.tile([1, QT], F32, tag="den", name=f"den{b}{h}{qt}")
                ecopy(den[:], avs[h][64:65, :])
                oraws.append((oraw, den))
            rcps = []
            for h in range(HPC):
                rcp = smallp.tile([1, QT], F32, tag="rcp", name=f"rcp{b}{h}{qt}")
                nc.vector.reciprocal_approx_fast(rcp[:], oraws[h][1][:])
                rcps.append(rcp)
            for h in range(HPC):
                # keep the PE out of the tail divide entirely (its queue is
                # occupied by recv-gated projection matmuls): DRAM-bounce
                # broadcast, one head per free DMA ring
                ring = nc.sync if h == 0 else nc.scalar
                rcp_d = dramp.tile([1, QT], F32, tag="rcpd", bufs=4,
                                   name=f"rcpdL{b}{h}{qt}")
                ring.dma_start(out=rcp_d[:], in_=rcps[h][:])
                bcast = bass.AP(tensor=rcp_d.tensor, offset=rcp_d.offset,
                                ap=[[0, 64]] + list(rcp_d.ap[1:]))
                bcs = smallp.tile([64, QT], F32, tag="bcs", name=f"bcs{b}{h}{qt}")
                ring.dma_start(out=bcs[:], in_=bcast)
                bcss.append(bcs)
            for h in range(HPC):
                odiv = smallp.tile([64, QT], BF16, tag="odiv",
                                   name=f"odiv{b}{h}{qt}")
                nc.vector.tensor_mul(odiv[:], oraws[h][0][:], bcss[h][:])
                send(h, odiv)
            return

        for h in range(HPC):
            oraw = smallp.tile([64, QT], F32, tag="oraw", name=f"oraw{b}{h}{qt}")
            nc.vector.tensor_copy(oraw[:], avs[h][0:64, :])
            den = smallp.tile([1, QT], F32, tag="den", name=f"den{b}{h}{qt}")
            nc.vector.tensor_copy(den[:], avs[h][64:65, :])
            rcp = smallp.tile([1, QT], F32, tag="rcp", name=f"rcp{b}{h}{qt}")
            # reciprocal_approx_fast is a custom DVE op: in/out must sit at
            # base partition 0, hence the separate denominator eviction
            nc.vector.reciprocal_approx_fast(rcp[:], den[:])
            bcs = smallp.tile([64, QT], F32, tag="bcs", name=f"bcs{b}{h}{qt}")
            rcp_d = dramp.tile([1, QT], F32, tag="rcpd", bufs=4,
                               name=f"rcpd{b}{h}{qt}")
            nc.sync.dma_start(out=rcp_d[:], in_=rcp[:])
            bcast = bass.AP(tensor=rcp_d.tensor, offset=rcp_d.offset,
                            ap=[[0, 64]] + list(rcp_d.ap[1:]))
            nc.sync.dma_start(out=bcs[:], in_=bcast)
            odiv = smallp.tile([64, QT], BF16, tag="odiv", name=f"odiv{b}{h}{qt}")
            nc.vector.tensor_mul(odiv[:], oraw[:], bcs[:])
            send(h, odiv)

    def emit_attention_batch(b, sched):
        """All 4 q-tiles of a batch as one rolling pipeline over 64+LAG
        (qt, kc) units: scores+exp lead, attn@v trails by LAG units, the
        divide chain fires as each q-tile's accumulation completes. sched
        maps unit -> list of dribble closures popped at that unit's top."""
        scale = 1.0 / math.sqrt(HD)
        NU = N_QT * N_KC
        LAG = 5
        pts = {}
        avs = {}
        for u in range(NU + LAG):
            for chain in sched.pop(u, ()):
                chain()
            if u < NU:
                qt, kc = divmod(u, N_KC)
                if kc == 0:
                    avs[qt] = [ps_av.tile([65, QT], F32, tag="av",
                                          name=f"av{b}{h}{qt}")
                               for h in range(HPC)]
                sps = ps_sps.tile([128, 2 * QT], F32, tag="sps",
                                  name=f"s{b}{qt}_{kc}")
                for h in range(HPC):
                    hof = h * 64
                    nc.tensor.matmul(
                        sps[:, h * QT:(h + 1) * QT],
                        kT_sb[b][hof:hof + 64, kc * KC:(kc + 1) * KC],
                        qT_sb[b][hof:hof + 64, qt * QT:(qt + 1) * QT],
                        start=True, stop=True)
                pt = pp.tile([128, 2 * QT], F32R, tag="p", name=f"p{b}{qt}_{kc}")
                nc.scalar.activation(pt[:], sps[:], EXPF, scale=scale)
                pts[u] = pt
            if u >= LAG:
                j = u - LAG
                qt2, kc2 = divmod(j, N_KC)
                for h in range(HPC):
                    nc.tensor.matmul(avs[qt2][h][:], v_sb[(b, h, kc2)][:],
                                     pts[j][:, h * QT:(h + 1) * QT],
                                     start=(kc2 == 0), stop=(kc2 == N_KC - 1))
                del pts[j]
                if kc2 == N_KC - 1:
                    emit_divide(b, qt2, avs.pop(qt2))
                    if qt2 == 1:
                        emit_a2a(b, 0)
        # leftovers (if the schedule ran past the unit count)
        for u in sorted(sched):
            for chain in sched.pop(u):
                chain()

    def emit_a2a(b, hf):
        nc.gpsimd.collective_compute(
            "AllToAll", mybir.AluOpType.bypass,
            replica_groups=[list(range(N_CORES))],
            ins=[send_d[(b, hf)].opt()], outs=[recv_d[(b, hf)].opt()])

    # ---------------- emission ----------------
    # batch-0 r-tile 0: k then q inline (scores(qt0, kc0..3) ready ASAP)
    for chain in qk_chain("k", 0, dribbled=False):
        chain()
    for chain in qk_chain("q", 0, dribbled=False):
        chain()

    # batch-0 dribble: deadline-scheduled remaining batch-0 chains (all
    # consumers of xt_n emitted before any consumer of xt_{n+2}, since the
    # xtp pool has 2 buffers), then batch-1 qkv from u=24 (1 pop/unit)
    sched = {}
    def put(u, *chains):
        sched.setdefault(u, []).extend(chains)
    v0 = v_chains(0, dribbled=True)
    k1 = qk_chain("k", 1, dribbled=True)
    v1 = v_chains(1, dribbled=True)
    q1 = qk_chain("q", 1, dribbled=True)
    k2 = qk_chain("k", 2, dribbled=True)
    k3 = qk_chain("k", 3, dribbled=True)
    v2 = v_chains(2, dribbled=True)
    q2 = qk_chain("q", 2, dribbled=True)
    v3 = v_chains(3, dribbled=True)
    q3 = qk_chain("q", 3, dribbled=True)
    put(1, v0[0], v0[1])
    put(2, k1[0])
    put(3, k1[1])
    put(4, v0[2])
    put(5, v0[3])
    put(6, k2[0], v1[0])
    put(7, k2[1], v1[1])
    put(8, q1[0])
    put(9, v1[2], q1[1])
    put(10, v1[3])
    put(11, k3[0])
    put(12, k3[1], v2[0])
    put(13, v2[1], v2[2])
    put(14, v2[3])
    put(15, q2[0])
    put(16, q2[1], v3[0])
    put(17, v3[1], v3[2])
    put(18, v3[3])
    put(19, q3[0])
    put(20, q3[1])
    # batch-1 qkv chains (xt loads posted now; gpsimd queue self-paces on
    # the xtp pool)
    b1_chains = []
    b1_late = []
    for rt in range(N_QT, 2 * N_QT):
        emit_xt_load(rt)
        b1_chains.extend(qk_chain("k", rt, dribbled=True))
        if rt < 2 * N_QT - 2:
            b1_chains.extend(qk_chain("q", rt, dribbled=True))
            b1_chains.extend(v_chains(rt, dribbled=True))
        else:
            # rt6/rt7's q/v ride in batch-1's own window (it has exp slack;
            # batch-0's window is PE-bound)
            b1_late.append(("q", rt))
            b1_late.append(("v", rt))
    for i, chain in enumerate(b1_chains):
        put(24 + i, chain)
    # w2 load after the xt posts, split across both rings (2 MB each);
    # needed only by the output projections from mid-batch-1 on
    w2_sb[0] = consts.tile([128, NEC, E], BF16, tag="w2", name="w2_all")
    w2r = w2T.rearrange("(c p) f -> p c f", p=128)
    nc.gpsimd.dma_start(out=w2_sb[0][:, 0:4, :], in_=w2r[:, 0:4, :])
    nc.sync.dma_start(out=w2_sb[0][:, 4:NEC, :], in_=w2r[:, 4:NEC, :])
    emit_attention_batch(0, sched)
    emit_a2a(0, 1)                     # second half, fires at batch-0 end

    # both batch-0 halves complete early in batch-1 attention;
    # batch-1 half 0's A2A fires mid-batch, its projection runs at the tail
    sched = {}
    q6 = qk_chain("q", 6, dribbled=True)
    v6 = v_chains(6, dribbled=True)
    q7 = qk_chain("q", 7, dribbled=True)
    v7 = v_chains(7, dribbled=True)
    put(1, v6[0])
    put(2, v6[1])
    put(3, v6[2])      # b1 kc8,9 needed at u13
    put(4, v6[3])      # kc10,11 at u15
    put(5, v7[0])
    put(6, v7[1])
    put(7, v7[2])      # kc12,13 at u17
    put(8, v7[3])      # kc14,15 at u19
    put(9, q6[0])
    put(10, q6[1])     # scores qt2 at u32
    put(11, q7[0])
    put(12, q7[1])     # scores qt3 at u48
    emit_attention_batch(1, sched)
    # ALL output projections run in the tail: the first three overlap the
    # final collective's transfer, and batch-1's attention window no longer
    # stalls when an earlier AllToAll draws a slow fabric (40-68us observed)
    emit_a2a(1, 1)
    for bb, hf in ((0, 0), (0, 1), (1, 0), (1, 1)):
        for chain in proj_chains(bb, hf):
            chain()
    ctx.close()


def _host_prep(x, w1, w2):
    import ml_dtypes
    bf16 = ml_dtypes.bfloat16
    x = np.ascontiguousarray(np.asarray(x, dtype=np.float32))
    w1 = np.ascontiguousarray(np.asarray(w1, dtype=np.float32))
    w2 = np.ascontiguousarray(np.asarray(w2, dtype=np.float32))

    xT = np.ascontiguousarray(x.reshape(R, E).T.astype(bf16))  # [E, R] bf16
    w2T = np.ascontiguousarray(w2.T.astype(bf16))              # [E, E] bf16

    theta = 1.0 / (BASE ** (np.arange(0, HD, 2, dtype=np.float32) / HD))
    enc = np.arange(S, dtype=np.float32)[:, None] * theta[None, :]
    enc = np.repeat(enc, 2, axis=-1)                      # [s, 64]
    cos1 = np.cos(enc).T.astype(np.float32)               # [64, S]
    sin1 = np.sin(enc).T.astype(np.float32)
    cosT = np.ascontiguousarray(np.concatenate([cos1, cos1], axis=0))
    sinT = np.ascontiguousarray(np.concatenate([sin1, sin1], axis=0))

    m64 = np.zeros((HD, HD), dtype=np.float32)
    for i in range(HD // 2):
        m64[2 * i, 2 * i + 1] = -1.0
        m64[2 * i + 1, 2 * i] = 1.0
    m128 = np.zeros((128, 128), dtype=np.float32)
    m128[:64, :64] = m64
    m128[64:, 64:] = m64
    p2T = np.ascontiguousarray(m128.T)

    in_maps = []
    for c in range(N_CORES):
        hA, hB = HPC * c, HPC * c + 1
        def rows(base):
            return np.concatenate(
                [w1[base + hA * HD: base + (hA + 1) * HD, :],
                 w1[base + hB * HD: base + (hB + 1) * HD, :]], axis=0)
        in_maps.append({
            "xT": xT,
            "wqT": np.ascontiguousarray(rows(0).T.astype(bf16)),
            "wkT": np.ascontiguousarray(rows(E).T.astype(bf16)),
            "wvT": np.ascontiguousarray(rows(2 * E).T.astype(bf16)),
            "w2T": w2T,
            "cosT": cosT,
            "sinT": sinT,
            "p2T": p2T,
        })
    return in_maps


def kernel(x, w1, w2, _trace=False):
    if "nc" not in _COMPILED:
        _COMPILED["nc"] = _build_nc()
    nc = _COMPILED["nc"]
    in_maps = _host_prep(x, w1, w2)
    res = run_bass_kernel_spmd(nc, in_maps, core_ids=list(range(N_CORES)),
                               trace=_trace)
    _COMPILED["last_result"] = res
    # core c returns [512, E] as four 128-row blocks:
    # [b0 s=128c.., b0 s=1024+128c.., b1 s=128c.., b1 s=1024+128c..]
    full = np.empty((B, S, E), dtype=np.float32)
    for c in range(N_CORES):
        blk = res.results[c]["out"]
        full[0, 128 * c:128 * (c + 1)] = blk[0:128]
        full[0, 1024 + 128 * c:1024 + 128 * (c + 1)] = blk[128:256]
        full[1, 128 * c:128 * (c + 1)] = blk[256:384]
        full[1, 1024 + 128 * c:1024 + 128 * (c + 1)] = blk[384:512]
    return full



# revision 4
# speedup vs baseline: 1.2739x; 1.2739x over previous
"""Trainium2 Bass kernel for nn_Attention_12000138625343.

Full multi-head attention layer (B=2, S=2048, E=1024, H=16, hd=64, interleaved
RoPE on q/k, non-causal softmax) run tensor-parallel over 8 NeuronCores:

  - heads sharded 2-per-core (w1 columns / qkv projection sharded),
  - x replicated and host-cast to bf16, passed pre-transposed [E, B*S],
  - q/k stored bf16 after RoPE; scores computed transposed [k, q] in bf16
    with the two heads' K=64 matmuls packed into disjoint PE row-groups,
  - exp on ACT outputs bf16 probabilities; v is projected transposed,
    PE-transposed back (bf16 identity) into [k, hd+1] tiles whose ones
    column accumulates the softmax denominator during attn@v,
  - the divide runs off the TensorEngine (DVE reciprocal + DRAM-bounce
    broadcast + DVE multiply),
  - NO collectives: after each q-tile's divide, the core immediately runs
    its partial output projection (contraction over its own 128 channels,
    all 4096 rows) and DMAs the bf16 partial rows out; the host sums the
    8 cores' partials (the unshard step). This removes the 4 AllToAlls
    (~25-35us each) and their tail exposure entirely,
  - qkv chains for later r-tiles are dribbled into the attention unit
    loop on a deadline schedule (consumers of x tile n before tile n+3).

Measured on the fixture: see test.py; rel err ~1e-2 vs the 2e-2 gate.
"""

import math

import numpy as np

import concourse.bass as bass
import concourse.mybir as mybir
import concourse.tile as tile
from concourse import bacc
from concourse.bass_utils import run_bass_kernel_spmd
from concourse.masks import make_identity

B, S, E, H = 2, 2048, 1024, 16
HD = E // H  # 64
BASE = 10000.0
N_CORES = 8
HPC = H // N_CORES       # heads per core = 2
R = B * S                # 4096 flattened rows
RT = 512                 # rows per r-tile
NEC = E // 128           # 8 e-chunks of 128
QT = 512                 # q columns per q-tile
N_QT = S // QT           # 4 q-tiles per batch
KC = 128                 # k rows per k-chunk
N_KC = S // KC           # 16 k-chunks per batch

F32 = mybir.dt.float32
F32R = mybir.dt.float32r
BF16 = mybir.dt.bfloat16
EXPF = mybir.ActivationFunctionType.Exp

_COMPILED = {}


def _build_nc():
    nc = bacc.Bacc("TRN2", target_bir_lowering=False, debug=False,
                   num_devices=N_CORES)

    xT = nc.dram_tensor("xT", [E, R], BF16, kind="ExternalInput").ap()
    wqT = nc.dram_tensor("wqT", [E, 128], BF16, kind="ExternalInput").ap()
    wkT = nc.dram_tensor("wkT", [E, 128], BF16, kind="ExternalInput").ap()
    wvT = nc.dram_tensor("wvT", [E, 128], BF16, kind="ExternalInput").ap()
    w2my = nc.dram_tensor("w2my", [128, E], BF16, kind="ExternalInput").ap()
    cosT = nc.dram_tensor("cosT", [128, S], F32, kind="ExternalInput").ap()
    sinT = nc.dram_tensor("sinT", [128, S], F32, kind="ExternalInput").ap()
    p2T = nc.dram_tensor("p2T", [128, 128], F32, kind="ExternalInput").ap()
    out = nc.dram_tensor("out", [R, E], BF16, kind="ExternalOutput").ap()

    with tile.TileContext(nc) as tc:
        _emit(tc, nc, xT, wqT, wkT, wvT, w2my, cosT, sinT, p2T, out)
    nc.compile()
    return nc


def _emit(tc, nc, xT, wqT, wkT, wvT, w2my, cosT, sinT, p2T, out):
    import contextlib
    ctx = contextlib.ExitStack()
    consts = ctx.enter_context(tc.tile_pool(name="consts", bufs=1))
    xtp = ctx.enter_context(tc.tile_pool(name="xtp", bufs=3))
    qkp = ctx.enter_context(tc.tile_pool(name="qkp", bufs=1))
    rawp = ctx.enter_context(tc.tile_pool(name="rawp", bufs=2))
    tmpp = ctx.enter_context(tc.tile_pool(name="tmpp", bufs=2))
    vp = ctx.enter_context(tc.tile_pool(name="vp", bufs=1))
    pp = ctx.enter_context(tc.tile_pool(name="pp", bufs=7))
    smallp = ctx.enter_context(tc.tile_pool(name="smallp", bufs=2))
    otp = ctx.enter_context(tc.tile_pool(name="otp", bufs=3))
    dramp = ctx.enter_context(tc.tile_pool(name="dramp", bufs=1, space="DRAM"))
    # PSUM budget (8 banks): qkv/proj 2 + sps 2 x 2 + av 2 = 8
    ps_qkv = ctx.enter_context(tc.tile_pool(name="ps_qkv", bufs=2, space="PSUM"))
    ps_sps = ctx.enter_context(tc.tile_pool(name="ps_sps", bufs=2, space="PSUM"))
    ps_av = ctx.enter_context(tc.tile_pool(name="ps_av", bufs=2, space="PSUM"))

    # ---- tiny constants first: the identity (gpsimd iota) must precede the
    # xt posts on the gpsimd ring or the v-transposes deadlock against a
    # blocked xt DMA ----
    ones_f32 = consts.tile([128, 64], F32, tag="ones32", name="ones_f32")
    nc.vector.memset(ones_f32[:], 1.0)
    ones_bf = consts.tile([128, 1], BF16, tag="onesb", name="ones_bf")
    nc.vector.tensor_copy(ones_bf[:], ones_f32[:, 0:1])
    id_sb = consts.tile([128, 128], F32, tag="idm", name="id_sb")
    make_identity(nc, id_sb[:])
    id_bf = consts.tile([128, 128], BF16, tag="idb", name="id_bf")
    nc.vector.tensor_copy(id_bf[:], id_sb[:])

    # ---- weight/x loads, k-chain inputs first so scores can start early.
    # Each batch-0 x r-tile is split across the DMA queues so multiple rings
    # pull HBM concurrently; cos/sin are loaded per 512-column chunk just in
    # time for each r-tile's RoPE ----
    xTr = xT.rearrange("(c p) r -> p c r", p=128)
    wk_all = consts.tile([128, NEC, 128], BF16, tag="wk", name="wk_all")
    nc.gpsimd.dma_start(out=wk_all[:], in_=wkT.rearrange("(c p) f -> p c f", p=128))
    wq_all = consts.tile([128, NEC, 128], BF16, tag="wq", name="wq_all")
    nc.sync.dma_start(out=wq_all[:],
                  in_=wqT.rearrange("(c p) f -> p c f", p=128))
    xts = {}
    cos_sb = consts.tile([128, S], F32, tag="cos", name="cos_sb")
    sin_sb = consts.tile([128, S], F32, tag="sin", name="sin_sb")
    p2_sb = consts.tile([128, 128], F32R, tag="p2", name="p2_sb")
    wv_all = consts.tile([128, NEC, 128], BF16, tag="wv", name="wv_all")
    w2_sb = consts.tile([128, E], BF16, tag="w2", name="w2_sb")

    def post_xt_split(rt, four=False):
        t = xtp.tile([128, NEC, RT], BF16, tag="xt", name=f"xt_{rt}")
        c0, c1 = rt * RT, (rt + 1) * RT
        if four:
            # first tiles gate the whole pipeline: pull them over several
            # DGE rings at once (each ring drains its descriptors serially)
            nc.gpsimd.dma_start(out=t[:, 0:3, :], in_=xTr[:, 0:3, c0:c1])
            nc.sync.dma_start(out=t[:, 3:5, :], in_=xTr[:, 3:5, c0:c1])
            nc.scalar.dma_start(out=t[:, 5:NEC, :], in_=xTr[:, 5:NEC, c0:c1])
        else:
            nc.gpsimd.dma_start(out=t[:, 0:4, :], in_=xTr[:, 0:4, c0:c1])
            nc.sync.dma_start(out=t[:, 4:NEC, :], in_=xTr[:, 4:NEC, c0:c1])
        xts[rt] = t
        return t

    def cossin(i):
        nc.gpsimd.dma_start(out=cos_sb[:, i * RT:(i + 1) * RT],
                            in_=cosT[:, i * RT:(i + 1) * RT])
        nc.gpsimd.dma_start(out=sin_sb[:, i * RT:(i + 1) * RT],
                            in_=sinT[:, i * RT:(i + 1) * RT])

    # sync ring carries only x halves (plus wq/p2): both DGE rings process
    # their descriptors serially, so RoPE tables must not delay x tiles
    post_xt_split(0, four=True)
    nc.sync.dma_start(out=p2_sb[:], in_=p2T[:, :].bitcast(F32R))
    nc.gpsimd.dma_start(out=wv_all[:], in_=wvT.rearrange("(c p) f -> p c f", p=128))
    cossin(0)
    post_xt_split(1, four=True)
    cossin(1)
    post_xt_split(2)
    cossin(2)
    post_xt_split(3)
    cossin(3)
    # w2 slice (256KB) on the scalar ring; needed from the first divide on
    nc.scalar.dma_start(out=w2_sb[:], in_=w2my[:, :].rearrange("p f -> p f"))

    qT_sb, kT_sb, v_sb = {}, {}, {}

    def emit_xt_load(rt):
        # batch-1 tiles load on gpsimd only: their posts block on xtp pool
        # reuse, and the sync queue must stay clear for the divide DMAs
        if rt in xts:
            return xts[rt]
        t = xtp.tile([128, NEC, RT], BF16, tag="xt", name=f"xt_{rt}")
        nc.gpsimd.dma_start(out=t[:], in_=xTr[:, :, rt * RT:(rt + 1) * RT])
        xts[rt] = t
        return t

    def qk_chain(kind, rt, dribbled):
        """Two closures emitting the q- or k-projection (+RoPE) for r-tile
        rt. Dribbled chains evict on DVE to keep ACT free for exp."""
        b, st = rt // N_QT, (rt % N_QT) * RT
        w_all = wq_all if kind == "q" else wk_all
        if b not in qT_sb:
            qT_sb[b] = qkp.tile([128, S], BF16, tag=f"qT{b}", name=f"qT{b}")
            kT_sb[b] = qkp.tile([128, S], BF16, tag=f"kT{b}", name=f"kT{b}")
        dst = qT_sb[b] if kind == "q" else kT_sb[b]
        state = {}

        def emit_a():
            xt = xts[rt]
            acc = ps_qkv.tile([128, RT], F32, tag="qkv", name=f"{kind}acc{rt}")
            for ec in range(4):
                nc.tensor.matmul(acc[:], w_all[:, ec, :], xt[:, ec, :],
                                 start=(ec == 0), stop=False)
            state["acc"] = acc

        def emit_b():
            xt = xts[rt]
            acc = state.pop("acc")
            for ec in range(4, NEC):
                nc.tensor.matmul(acc[:], w_all[:, ec, :], xt[:, ec, :],
                                 start=False, stop=(ec == NEC - 1))
            raw = rawp.tile([128, RT], F32R, tag="raw", name=f"{kind}raw{rt}")
            if dribbled:
                nc.vector.tensor_copy(raw[:], acc[:])
            else:
                nc.scalar.copy(raw[:], acc[:])
            rot = ps_qkv.tile([128, RT], F32, tag="qkv", name=f"{kind}rot{rt}")
            nc.tensor.matmul(rot[:], p2_sb[:], raw[:], start=True, stop=True)
            t1 = tmpp.tile([128, RT], F32, tag="ropet", name=f"{kind}t1_{rt}")
            nc.vector.tensor_mul(t1[:], raw[:].bitcast(F32),
                                 cos_sb[:, st:st + RT])
            t2 = tmpp.tile([128, RT], F32, tag="ropet", name=f"{kind}t2_{rt}")
            nc.vector.tensor_mul(t2[:], rot[:], sin_sb[:, st:st + RT])
            nc.vector.tensor_add(dst[:, st:st + RT], t1[:], t2[:])
        return [emit_a, emit_b]

    def v_chains(rt, dribbled):
        """Four closures for the v projection of r-tile rt: two matmul halves
        in transposed orientation, two transpose-back pairs."""
        b = rt // N_QT
        vstate = {}

        def head(half):
            def emit():
                xt = xts[rt]
                if half == 0:
                    vacc = ps_qkv.tile([128, RT], F32, tag="qkv",
                                       name=f"vTacc{rt}")
                    vstate["ps"] = vacc
                vacc = vstate["ps"]
                for ec in range(4 * half, 4 * half + 4):
                    nc.tensor.matmul(vacc[:], wv_all[:, ec, :], xt[:, ec, :],
                                     start=(ec == 0), stop=(ec == NEC - 1))
                if half == 1:
                    vts = rawp.tile([128, RT], BF16, tag="rawb", name=f"vts{rt}")
                    if dribbled:
                        nc.vector.tensor_copy(vts[:], vstate.pop("ps")[:])
                    else:
                        nc.scalar.copy(vts[:], vstate.pop("ps")[:])
                    vstate["sb"] = vts
            return emit

        def tail(pair):
            def emit():
                vts = vstate["sb"]
                for sub in (2 * pair, 2 * pair + 1):
                    vtr = ps_qkv.tile([128, 128], BF16, tag="qkv",
                                      name=f"vtr{rt}_{sub}")
                    nc.tensor.transpose(
                        vtr[:], vts[:, sub * 128:(sub + 1) * 128], id_bf[:])
                    kc = (rt % N_QT) * 4 + sub
                    for h in range(HPC):
                        vt = vp.tile([128, 65], BF16, tag=f"v{b}{h}{kc}",
                                     name=f"v{b}{h}{kc}")
                        nc.vector.tensor_copy(vt[:, 0:64],
                                              vtr[:, h * 64:(h + 1) * 64])
                        nc.vector.tensor_copy(vt[:, 64:65], ones_bf[:, 0:1])
                        v_sb[(b, h, kc)] = vt
            return emit

        return [head(0), head(1), tail(0), tail(1)]

    def emit_divide(b, qt, avs):
        """Divide by the softmax denominator (row 64 of av), then run this
        q-tile's partial output projection (contraction over my 128 chans)
        and DMA the bf16 partial rows out. PE-free divide: fast DVE
        reciprocal + DRAM-bounce broadcast DMA."""
        odiv = smallp.tile([128, QT], BF16, tag="odiv", name=f"odiv{b}{qt}")
        for h in range(HPC):
            oraw = smallp.tile([64, QT], F32, tag="oraw", name=f"oraw{b}{h}{qt}")
            nc.vector.tensor_copy(oraw[:], avs[h][0:64, :])
            den = smallp.tile([1, QT], F32, tag="den", name=f"den{b}{h}{qt}")
            nc.vector.tensor_copy(den[:], avs[h][64:65, :])
            rcp = smallp.tile([1, QT], F32, tag="rcp", name=f"rcp{b}{h}{qt}")
            # reciprocal_approx_fast is a custom DVE op: in/out must sit at
            # base partition 0, hence the separate denominator eviction
            nc.vector.reciprocal_approx_fast(rcp[:], den[:])
            bcs = smallp.tile([64, QT], F32, tag="bcs", name=f"bcs{b}{h}{qt}")
            rcp_d = dramp.tile([1, QT], F32, tag="rcpd", bufs=4,
                               name=f"rcpd{b}{h}{qt}")
            ring = nc.sync if h == 0 else nc.scalar
            ring.dma_start(out=rcp_d[:], in_=rcp[:])
            bcast = bass.AP(tensor=rcp_d.tensor, offset=rcp_d.offset,
                            ap=[[0, 64]] + list(rcp_d.ap[1:]))
            ring.dma_start(out=bcs[:], in_=bcast)
            nc.vector.tensor_mul(odiv[h * 64:(h + 1) * 64, :],
                                 oraw[:], bcs[:])

        # partial projection: out rows [b*2048 + qt*512 + rb*128, :]
        def proj(rb):
            def emit():
                ot = otp.tile([128, E], BF16, tag="ot", name=f"ot{b}{qt}_{rb}")
                for fh in range(2):
                    ops = ps_qkv.tile([128, RT], F32, tag="qkv",
                                      name=f"ops{b}{qt}_{rb}_{fh}")
                    nc.tensor.matmul(
                        ops[:],
                        odiv[:, rb * 128:(rb + 1) * 128],
                        w2_sb[:, fh * 512:(fh + 1) * 512],
                        start=True, stop=True)
                    nc.vector.tensor_copy(ot[:, fh * 512:(fh + 1) * 512],
                                          ops[:])
                r0 = b * S + qt * QT + rb * 128
                nc.scalar.dma_start(out=out[r0:r0 + 128, :], in_=ot[:])
            return emit
        return [proj(rb) for rb in range(4)]

    def emit_attention_batch(b, sched):
        """All 4 q-tiles of a batch as one rolling pipeline over 64+LAG
        (qt, kc) units: scores+exp lead, attn@v trails by LAG units, the
        divide chain fires as each q-tile's accumulation completes. sched
        maps unit -> list of dribble closures popped at that unit's top."""
        scale = 1.0 / math.sqrt(HD)
        NU = N_QT * N_KC
        LAG = 5
        pts = {}
        avs = {}
        for u in range(NU + LAG):
            for chain in sched.pop(u, ()):
                chain()
            if u < NU:
                qt, kc = divmod(u, N_KC)
                if kc == 0:
                    avs[qt] = [ps_av.tile([65, QT], F32, tag="av",
                                          name=f"av{b}{h}{qt}")
                               for h in range(HPC)]
                sps = ps_sps.tile([128, 2 * QT], F32, tag="sps",
                                  name=f"s{b}{qt}_{kc}")
                for h in range(HPC):
                    hof = h * 64
                    nc.tensor.matmul(
                        sps[:, h * QT:(h + 1) * QT],
                        kT_sb[b][hof:hof + 64, kc * KC:(kc + 1) * KC],
                        qT_sb[b][hof:hof + 64, qt * QT:(qt + 1) * QT],
                        start=True, stop=True)
                pt = pp.tile([128, 2 * QT], BF16, tag="p", name=f"p{b}{qt}_{kc}")
                nc.scalar.activation(pt[:], sps[:], EXPF, scale=scale)
                pts[u] = pt
            if u >= LAG:
                j = u - LAG
                qt2, kc2 = divmod(j, N_KC)
                for h in range(HPC):
                    nc.tensor.matmul(avs[qt2][h][:], v_sb[(b, h, kc2)][:],
                                     pts[j][:, h * QT:(h + 1) * QT],
                                     start=(kc2 == 0), stop=(kc2 == N_KC - 1))
                del pts[j]
                if kc2 == N_KC - 1:
                    for chain in emit_divide(b, qt2, avs.pop(qt2)):
                        chain()
        # leftovers (if the schedule ran past the unit count)
        for u in sorted(sched):
            for chain in sched.pop(u):
                chain()

    # ---------------- emission ----------------
    # batch-0 r-tile 0: k then q inline (scores(qt0, kc0..3) ready ASAP)
    for chain in qk_chain("k", 0, dribbled=False):
        chain()
    for chain in qk_chain("q", 0, dribbled=False):
        chain()

    # batch-0 dribble: deadline-scheduled remaining batch-0 chains (all
    # consumers of xt_n emitted before any consumer of xt_{n+3}, since the
    # xtp pool has 3 buffers), then batch-1 qkv from u=24 (1 pop/unit)
    sched = {}
    def put(u, *chains):
        sched.setdefault(u, []).extend(chains)
    v0 = v_chains(0, dribbled=True)
    k1 = qk_chain("k", 1, dribbled=True)
    v1 = v_chains(1, dribbled=True)
    q1 = qk_chain("q", 1, dribbled=True)
    k2 = qk_chain("k", 2, dribbled=True)
    k3 = qk_chain("k", 3, dribbled=True)
    v2 = v_chains(2, dribbled=True)
    q2 = qk_chain("q", 2, dribbled=True)
    v3 = v_chains(3, dribbled=True)
    q3 = qk_chain("q", 3, dribbled=True)
    put(1, v0[0], v0[1])
    put(2, k1[0])
    put(3, k1[1])
    put(4, v0[2])
    put(5, v0[3])
    put(6, k2[0], v1[0])
    put(7, k2[1], v1[1])
    put(8, q1[0])
    put(9, v1[2], q1[1])
    put(10, v1[3])
    put(11, k3[0])
    put(12, k3[1], v2[0])
    put(13, v2[1], v2[2])
    put(14, v2[3])
    put(15, q2[0])
    put(16, q2[1], v3[0])
    put(17, v3[1], v3[2])
    put(18, v3[3])
    put(19, q3[0])
    put(20, q3[1])
    # batch-1 qkv chains (xt loads posted now; gpsimd queue self-paces on
    # the xtp pool)
    b1_chains = []
    for rt in range(N_QT, 2 * N_QT):
        emit_xt_load(rt)
        b1_chains.extend(qk_chain("k", rt, dribbled=True))
        if rt < 2 * N_QT - 2:
            b1_chains.extend(qk_chain("q", rt, dribbled=True))
            b1_chains.extend(v_chains(rt, dribbled=True))
    for i, chain in enumerate(b1_chains):
        put(24 + i, chain)
    emit_attention_batch(0, sched)

    # rt6/rt7's q/v ride in batch-1's own window (it has exp slack;
    # batch-0's window is PE-bound)
    sched = {}
    q6 = qk_chain("q", 6, dribbled=True)
    v6 = v_chains(6, dribbled=True)
    q7 = qk_chain("q", 7, dribbled=True)
    v7 = v_chains(7, dribbled=True)
    put(1, v6[0])
    put(2, v6[1])
    put(3, v6[2])      # b1 kc8,9 needed at u13
    put(4, v6[3])      # kc10,11 at u15
    put(5, v7[0])
    put(6, v7[1])
    put(7, v7[2])      # kc12,13 at u17
    put(8, v7[3])      # kc14,15 at u19
    put(9, q6[0])
    put(10, q6[1])     # scores qt2 at u32
    put(11, q7[0])
    put(12, q7[1])     # scores qt3 at u48
    emit_attention_batch(1, sched)
    ctx.close()


def _host_prep(x, w1, w2):
    import ml_dtypes
    bf16 = ml_dtypes.bfloat16
    x = np.ascontiguousarray(np.asarray(x, dtype=np.float32))
    w1 = np.ascontiguousarray(np.asarray(w1, dtype=np.float32))
    w2 = np.ascontiguousarray(np.asarray(w2, dtype=np.float32))

    xT = np.ascontiguousarray(x.reshape(R, E).T.astype(bf16))  # [E, R] bf16

    theta = 1.0 / (BASE ** (np.arange(0, HD, 2, dtype=np.float32) / HD))
    enc = np.arange(S, dtype=np.float32)[:, None] * theta[None, :]
    enc = np.repeat(enc, 2, axis=-1)                      # [s, 64]
    cos1 = np.cos(enc).T.astype(np.float32)               # [64, S]
    sin1 = np.sin(enc).T.astype(np.float32)
    cosT = np.ascontiguousarray(np.concatenate([cos1, cos1], axis=0))
    sinT = np.ascontiguousarray(np.concatenate([sin1, sin1], axis=0))

    m64 = np.zeros((HD, HD), dtype=np.float32)
    for i in range(HD // 2):
        m64[2 * i, 2 * i + 1] = -1.0
        m64[2 * i + 1, 2 * i] = 1.0
    m128 = np.zeros((128, 128), dtype=np.float32)
    m128[:64, :64] = m64
    m128[64:, 64:] = m64
    p2T = np.ascontiguousarray(m128.T)

    in_maps = []
    for c in range(N_CORES):
        e0 = 128 * c           # heads 2c, 2c+1 occupy chans [128c, 128c+128)
        in_maps.append({
            "xT": xT,
            "wqT": np.ascontiguousarray(w1[e0:e0 + 128, :].T.astype(bf16)),
            "wkT": np.ascontiguousarray(w1[E + e0:E + e0 + 128, :].T.astype(bf16)),
            "wvT": np.ascontiguousarray(
                w1[2 * E + e0:2 * E + e0 + 128, :].T.astype(bf16)),
            "w2my": np.ascontiguousarray(w2[:, e0:e0 + 128].T.astype(bf16)),
            "cosT": cosT,
            "sinT": sinT,
            "p2T": p2T,
        })
    return in_maps


def kernel(x, w1, w2, _trace=False):
    if "nc" not in _COMPILED:
        _COMPILED["nc"] = _build_nc()
    nc = _COMPILED["nc"]
    in_maps = _host_prep(x, w1, w2)
    res = run_bass_kernel_spmd(nc, in_maps, core_ids=list(range(N_CORES)),
                               trace=_trace)
    _COMPILED["last_result"] = res
    # each core returns its partial projection over its own 128 channels
    # for all 4096 rows; the full output is the sum of the 8 partials
    acc = np.zeros((R, E), dtype=np.float32)
    for c in range(N_CORES):
        acc += res.results[c]["out"].astype(np.float32)
    return acc.reshape(B, S, E)
